# revision 1
# baseline (speedup 1.0000x reference)
"""Trainium2 Bass kernel for nn_DepthwiseMultiKernelAttention.

Reference computes:
    dw  = depthwise3x3(x, K)              (pad=1, stride=1, per-channel)
    out = softmax_rows(G) @ P @ softmax_rows(A) @ dw     (channel mixes)

All three channel mixes are linear maps on the channel dim, so they fold into
a single 64x64 matrix  M = G_sm @ P @ A_sm.  Folding M through the depthwise
conv turns the whole module into nine shifted channel-mix matmuls:

    out[:, h, w] = sum_{dy,dx} M9[dy,dx] @ x[:, h+dy-1, w+dx-1]
    M9[dy,dx][j, c] = M[j, c] * K[c, dy, dx]

Sharding (8 cores): data-parallel, core i = (sample pair i//2, row half i%2).
Each core packs (2 samples x 64 ch) into the 128 SBUF partitions; the host
pre-pads each shard with the 1-pixel zero/halo border so the device kernel
has no edge cases.

Per-core device schedule (all engines near-saturated, ~104us modeled):
  - TensorE: per 2-row PSUM tile, 5-6 shift matmuls (fp32r, N=512 free dim,
    out = blockdiag(M9[s].T, M9[s].T).T @ x128[shift s]) accumulated in PSUM,
    plus one blockdiag(M.T, M.T) "mix" matmul folding in the vector-path
    partial sum.  The mix matmul is deferred one tile to give VectorE slack.
  - ScalarE + VectorE: 3 of the 9 shift terms (si=6,7,8) are computed as
    per-channel scalar FMAs at 8-row granularity (ScalarE multiply, then two
    chained VectorE scalar_tensor_tensor FMAs); odd tiles of interior
    chunks offload a 4th term (si=5) as a 2-row FMA to balance PE vs DVE
    (boundary chunks keep it on PE so VectorE is off the critical path at
    the ends).  ScalarE also drains PSUM->SBUF and copies the 2 halo rows
    between chunk tiles.
  - DMA: inputs stream on the SP HWDGE queue in 16-row chunks (first chunk
    split 4/6/8 rows so PE and the vector path start early); only fresh rows
    are loaded (halo rows copied on-chip).  Stores run on the GpSimd SWDGE
    queue per 8 output rows (last chunk on SP, which is idle by then).
  - 16 dummy matmuls on a zeroed scratch tile fill the initial DMA wait so
    the PE HAM clock gate is warm when the first real matmul issues.
HBM traffic is ~32.4 MB/core vs a ~91us roofline at 358 GB/s; the cost-model
timeline is ~104us with DMA 92%, PE 93%, DVE 81% busy.
"""

import numpy as np

B, C, H, W = 8, 64, 256, 256
N_CORES = 8
HH = H // 2  # rows per core (half image)
PR, PC = HH + 2, W + 2  # padded shard rows/cols (halo)
ROWS_PER_CHUNK = 16
N_CHUNKS = HH // ROWS_PER_CHUNK
ROWS_PER_PSUM = 2  # 2*256 = 512 fp32 = one PSUM bank
SHIFTS = [(dy, dx) for dy in range(3) for dx in range(3)]
N_VEC = 3  # shift terms offloaded to ScalarE/VectorE in hybrid mode
FOURTH_SI = 2  # extra shift folded on VectorE for odd tiles (dy=0 fits tile)
MIX = 6  # wt slot holding the plain blockdiag(M.T, M.T) mix matrix

# bf16 PE path: ~1.3x faster matmuls on silicon (fp32r self-loading weight
# reloads are not hidden) but ~15x looser numerics (2.8e-3 vs 1.9e-4 rel).
# fp32r is the default: the kernel is memory-bound either way and fp32r's
# accuracy comfortably clears any fp32-envelope check.
PE_BF16 = False

LAST_EXEC_NS = None
_PROGRAM = None


def _build_program(hybrid=True, psum_bufs=8, split_first_dma=True,
                   x_bufs=3, vec8=True, fourth=True, psum4=False,
                   o_bufs=3, acc28_bufs=3, pe_bf16=False, loop_iters=None):
    import concourse.mybir as mybir
    import concourse.tile as tile
    from concourse import bacc

    f32 = mybir.dt.float32
    f32r = mybir.dt.float32r
    mult = mybir.AluOpType.mult
    add = mybir.AluOpType.add
    Copy = mybir.ActivationFunctionType.Copy

    # compute dtype for the matmul path: bf16 streams with hidden weight
    # loads on real HW (fp32r self-loading matmuls measure ~1.3-1.8x slower)
    cdt = mybir.dt.bfloat16 if pe_bf16 else f32r
    kdt = f32  # ScalarE activation requires fp32 scale APs
    # bf16 mode: x is cast fp32->bf16 in-flight by SWDGE; inputs ride the
    # gpsimd queue and stores move to SP so neither blocks the other
    in_dma = (lambda **kw: nc.gpsimd.dma_start(**kw)) if pe_bf16 else (
        lambda **kw: nc.sync.dma_start(**kw))

    nc = bacc.Bacc(
        "TRN2", target_bir_lowering=False, debug=False, num_devices=N_CORES
    )
    x_d = nc.dram_tensor(
        "xs", [128, PR, PC], f32 if pe_bf16 else f32r, kind="ExternalInput"
    ).ap()
    w_d = nc.dram_tensor("wt", [128, 7, 128], cdt, kind="ExternalInput").ap()
    k_d = nc.dram_tensor("kv", [128, 9], kdt, kind="ExternalInput").ap()
    o_d = nc.dram_tensor("out", [128, HH, W], f32, kind="ExternalOutput").ap()

    n_pe_shifts = (9 - N_VEC) if hybrid else 9

    with tile.TileContext(nc) as tc:
        with (
            tc.tile_pool(name="wpool", bufs=1) as wpool,
            tc.tile_pool(name="xpool", bufs=x_bufs) as xpool,
            tc.tile_pool(name="ppool", bufs=psum_bufs, space="PSUM") as ppool,
            tc.tile_pool(name="opool", bufs=o_bufs) as opool,
            tc.tile_pool(name="vpool", bufs=6) as vpool,
        ):
            import contextlib

            loop_cm = (
                tc.For_i(0, loop_iters, 1)
                if loop_iters is not None
                else contextlib.nullcontext()
            )
            with loop_cm:
                # PE warm-up: dummy matmuls on a zeroed scratch tile fill the
                # initial DMA wait so the HAM clock gate / p-state is at full
                # rate when the first real matmul issues.
                scratch = wpool.tile([128, 512], f32, tag="scratch")
                nc.gpsimd.memset(scratch, 0.0)
                sc16 = scratch.bitcast(mybir.dt.bfloat16)
                wps = ppool.tile([128, ROWS_PER_PSUM, W], f32, name="ps", tag="ps")
                for _ in range(16):
                    nc.tensor.matmul(
                        wps, lhsT=sc16[:, :128], rhs=sc16[:, :512],
                        start=True, stop=True,
                    )
                first = xpool.tile(
                    [128, ROWS_PER_CHUNK + 2, PC], cdt, name="xt", tag="xt"
                )
                wt = wpool.tile([128, 7, 128], cdt)
                nc.scalar.dma_start(out=wt, in_=w_d)
                kv = wpool.tile([128, 9], kdt)
                nc.scalar.dma_start(out=kv, in_=k_d)
                if split_first_dma:
                    # weights first: PE's first real matmul starts sooner;
                    # chunk 0 uses fine 2-row vec chains (tile 0 needs only
                    # rows 2-3, in the first x piece) so early mixes aren't
                    # starved by the 8-row group-chain latency
                    in_dma(out=first[:, :4, :], in_=x_d[:, :4, :])
                    in_dma(out=first[:, 4:10, :], in_=x_d[:, 4:10, :])
                if split_first_dma:
                    in_dma(
                        out=first[:, 10:, :],
                        in_=x_d[:, 10 : ROWS_PER_CHUNK + 2, :],
                    )
                else:
                    in_dma(out=first, in_=x_d[:, : ROWS_PER_CHUNK + 2, :])

                # (ps, acc2|None, dst, store|None) awaiting mix + copy
                pending = []
                DEFER = 1

                def flush_one():
                    ps, acc2, dst, cp_src, store = pending.pop(0)
                    if acc2 is not None:
                        nc.tensor.matmul(
                            ps, lhsT=wt[:, MIX, :], rhs=acc2, start=False, stop=True
                        )
                    if dst is not None:
                        nc.scalar.copy(out=dst, in_=cp_src)
                    if store is not None:
                        dram_ap, sbuf_ap, eng = store
                        # SWDGE (gpsimd) ring keeps stores off the input FIFO;
                        # the very last store goes via SP HWDGE (idle by then)
                        eng.dma_start(out=dram_ap, in_=sbuf_ap)

                def flush_pending(limit=0):
                    while len(pending) > limit:
                        flush_one()

                xt = first
                prev_xt = None
                for chunk in range(N_CHUNKS):
                    if chunk > 0:
                        r0 = chunk * ROWS_PER_CHUNK
                        xt = xpool.tile(
                            [128, ROWS_PER_CHUNK + 2, PC], cdt, name="xt", tag="xt"
                        )
                        # halo rows come from the previous chunk tile (saves DMA)
                        nc.scalar.copy(
                            out=xt[:, 0:2, :],
                            in_=prev_xt[:, ROWS_PER_CHUNK : ROWS_PER_CHUNK + 2, :],
                        )
                        in_dma(
                            out=xt[:, 2:, :],
                            in_=x_d[:, r0 + 2 : r0 + ROWS_PER_CHUNK + 2, :],
                        )
                    ot = opool.tile([128, ROWS_PER_CHUNK, W], f32)
                    n_tiles = ROWS_PER_CHUNK // ROWS_PER_PSUM
                    acc8 = {}
                    if hybrid and vec8 and chunk > 0:
                        # 8-row vec groups: fewer, bigger ScalarE/VectorE ops
                        GR = 8
                        for g in range(ROWS_PER_CHUNK // GR):
                            rg = GR * g

                            def xs8(si):
                                dy, dx = SHIFTS[si]
                                return xt[:, rg + dy : rg + dy + GR, dx : dx + W]

                            tmp8 = vpool.tile([128, GR, W], cdt, bufs=2)
                            nc.scalar.activation(
                                out=tmp8, in_=xs8(6), func=Copy, scale=kv[:, 6:7]
                            )
                            acc18 = vpool.tile([128, GR, W], cdt, bufs=2)
                            nc.vector.scalar_tensor_tensor(
                                out=acc18, in0=xs8(7), scalar=kv[:, 7:8], in1=tmp8,
                                op0=mult, op1=add,
                            )
                            acc28 = vpool.tile([128, GR, W], cdt, bufs=acc28_bufs)
                            nc.vector.scalar_tensor_tensor(
                                out=acc28, in0=xs8(8), scalar=kv[:, 8:9], in1=acc18,
                                op0=mult, op1=add,
                            )
                            acc8[g] = acc28
                    ps4_cache = {}
                    for t8 in range(n_tiles):
                        rr = ROWS_PER_PSUM * t8

                        def xs(si):
                            dy, dx = SHIFTS[si]
                            return xt[:, rr + dy : rr + dy + ROWS_PER_PSUM, dx : dx + W]

                        if psum4:
                            t4, sub = divmod(t8, 2)
                            if sub == 0:
                                ps4_cache[t4] = ppool.tile(
                                    [128, 2 * ROWS_PER_PSUM, W], f32, name="ps4",
                                    tag="ps4",
                                )
                            ps = ps4_cache[t4][
                                :, ROWS_PER_PSUM * sub : ROWS_PER_PSUM * (sub + 1), :
                            ]
                        else:
                            ps = ppool.tile([128, ROWS_PER_PSUM, W], f32)
                        acc2 = None
                        if hybrid and vec8 and chunk > 0:
                            g, q = divmod(t8, 8 // ROWS_PER_PSUM)
                            acc2 = acc8[g][
                                :, ROWS_PER_PSUM * q : ROWS_PER_PSUM * (q + 1), :
                            ]
                        elif hybrid:
                            # ScalarE: tmp = k6 * x6 ; VectorE: chain FMAs
                            tmp = vpool.tile([128, ROWS_PER_PSUM, W], cdt, bufs=4)
                            nc.scalar.activation(
                                out=tmp, in_=xs(6), func=Copy, scale=kv[:, 6:7]
                            )
                            acc1 = vpool.tile([128, ROWS_PER_PSUM, W], cdt, bufs=4)
                            nc.vector.scalar_tensor_tensor(
                                out=acc1, in0=xs(7), scalar=kv[:, 7:8], in1=tmp,
                                op0=mult, op1=add,
                            )
                            acc2 = vpool.tile([128, ROWS_PER_PSUM, W], cdt, bufs=4)
                            nc.vector.scalar_tensor_tensor(
                                out=acc2, in0=xs(8), scalar=kv[:, 8:9], in1=acc1,
                                op0=mult, op1=add,
                            )
                        interior = 0 < chunk < N_CHUNKS - 1
                        skip5 = hybrid and fourth and interior and (t8 % 2 == 1)
                        if skip5:
                            acc3 = vpool.tile([128, ROWS_PER_PSUM, W], cdt, bufs=4)
                            nc.vector.scalar_tensor_tensor(
                                out=acc3, in0=xs(5), scalar=kv[:, 5:6], in1=acc2,
                                op0=mult, op1=add,
                            )
                            acc2 = acc3
                        if skip5:
                            pe_shifts = list(range(n_pe_shifts - 1))
                        else:
                            pe_shifts = list(range(n_pe_shifts))
                        for idx, si in enumerate(pe_shifts):
                            nc.tensor.matmul(
                                ps,
                                lhsT=wt[:, si, :],
                                rhs=xs(si),
                                start=(idx == 0),
                                stop=(not hybrid and idx == len(pe_shifts) - 1),
                            )
                        # defer this tile's mix-matmul + copy until after the
                        # next tiles' shift matmuls (gives VectorE slack
                        # before PE consumes acc2)
                        flush_pending(limit=DEFER - 1)
                        if psum4:
                            # copy fires on the second half of the 4-row psum tile
                            dst = (
                                None
                                if t8 % 2 == 0
                                else ot[:, rr - ROWS_PER_PSUM : rr + ROWS_PER_PSUM, :]
                            )
                            cp_src = None if t8 % 2 == 0 else ps4_cache[t8 // 2]
                        else:
                            dst = ot[:, rr : rr + ROWS_PER_PSUM, :]
                            cp_src = ps
                        store = None
                        last_chunk = chunk == N_CHUNKS - 1
                        grp = 2 if last_chunk else 8  # finer tail stores
                        gtiles = grp // ROWS_PER_PSUM
                        if (t8 + 1) % gtiles == 0:
                            h0 = grp * ((t8 + 1) // gtiles - 1)
                            r0 = chunk * ROWS_PER_CHUNK + h0
                            if pe_bf16:
                                eng = nc.sync
                            else:
                                eng = nc.sync if chunk >= 5 else nc.gpsimd
                            store = (o_d[:, r0 : r0 + grp, :], ot[:, h0 : h0 + grp, :], eng)
                        if hybrid:
                            pending.append((ps, acc2, dst, cp_src, store))
                        else:
                            if dst is not None:
                                nc.scalar.copy(out=dst, in_=cp_src)
                            if store is not None:
                                store[2].dma_start(out=store[0], in_=store[1])
                    prev_xt = xt
                flush_pending()
    nc.compile()
    return nc


def _get_program():
    global _PROGRAM
    if _PROGRAM is None:
        _PROGRAM = _build_program(pe_bf16=PE_BF16)
    return _PROGRAM


def _softmax_rows(a):
    a = a.astype(np.float64)
    a = np.exp(a - a.max(axis=1, keepdims=True))
    return a / a.sum(axis=1, keepdims=True)


def _make_weights(depthwise_weights, pointwise_weights, attention_weights,
                  global_attention_weight):
    A = _softmax_rows(np.asarray(attention_weights))
    G = _softmax_rows(np.asarray(global_attention_weight))
    P = np.asarray(pointwise_weights)[:, :, 0, 0].astype(np.float64)
    M = G @ P @ A  # (64, 64): out = M @ dw per pixel
    Kdw = np.asarray(depthwise_weights)[:, 0].astype(np.float64)  # (64, 3, 3)
    wt = np.zeros((128, 7, 128), np.float32)
    for si, (dy, dx) in enumerate(SHIFTS[:MIX]):
        # lhsT block = M9[s].T where M9[s][j,c] = M[j,c]*K[c,dy,dx]
        blk = (M.T * Kdw[:, dy, dx][:, None]).astype(np.float32)  # (c, j)
        wt[:C, si, :C] = blk
        wt[C:, si, C:] = blk
    mixT = M.T.astype(np.float32)
    wt[:C, MIX, :C] = mixT
    wt[C:, MIX, C:] = mixT
    # per-partition depthwise taps for the ScalarE/VectorE path
    kva = np.empty((128, 9), np.float32)
    for si, (dy, dx) in enumerate(SHIFTS):
        kva[:C, si] = Kdw[:, dy, dx]
        kva[C:, si] = Kdw[:, dy, dx]
    return wt, kva


def _make_shards(x):
    x = np.asarray(x, dtype=np.float32)
    shards = []
    for i in range(N_CORES):
        p, h = divmod(i, 2)
        buf = np.zeros((2, C, PR, PC), np.float32)
        r0 = HH * h - 1
        r1 = HH * h + HH + 1
        sr0, sr1 = max(r0, 0), min(r1, H)
        buf[:, :, sr0 - r0 : sr1 - r0, 1 : 1 + W] = x[2 * p : 2 * p + 2, :, sr0:sr1, :]
        shards.append(buf.reshape(128, PR, PC))
    return shards


def kernel(x, depthwise_weights, pointwise_weights, attention_weights,
           global_attention_weight):
    global LAST_EXEC_NS
    from concourse import bass_utils

    nc = _get_program()
    wt, kv = _make_weights(depthwise_weights, pointwise_weights,
                           attention_weights, global_attention_weight)
    if PE_BF16:
        import ml_dtypes

        wt = wt.astype(ml_dtypes.bfloat16)
    shards = _make_shards(x)
    in_maps = [{"xs": shards[i], "wt": wt, "kv": kv} for i in range(N_CORES)]

    res = bass_utils.run_bass_kernel_spmd(
        nc, in_maps, core_ids=list(range(N_CORES)), trace=False
    )
    LAST_EXEC_NS = res.exec_time_ns

    out = np.empty((B, C, H, W), np.float32)
    for i in range(N_CORES):
        p, h = divmod(i, 2)
        o = res.results[i]["out"].reshape(2, C, HH, W)
        out[2 * p : 2 * p + 2, :, HH * h : HH * h + HH, :] = o
    return out



# revision 22
# speedup vs baseline: 1.0624x; 1.0624x over previous
"""Trainium2 Bass kernel for nn_DepthwiseMultiKernelAttention.

Reference computes:
    dw  = depthwise3x3(x, K)              (pad=1, stride=1, per-channel)
    out = softmax_rows(G) @ P @ softmax_rows(A) @ dw     (channel mixes)

All three channel mixes fold into one 64x64 matrix M = G_sm @ P @ A_sm, and
folding M through the depthwise conv gives nine shifted channel-mix matmuls:
    out[:, h, w] = sum_{dy,dx} M9[dy,dx] @ x[:, h+dy-1, w+dx-1]
    M9[dy,dx][j, c] = M[j, c] * K[c, dy, dx]

v2: everything is bf16 on the wire and on-chip. The host casts x to bf16
before upload and the output back to fp32 after download, halving HBM traffic
(the memory roofline) vs the fp32 baseline. Five of the nine taps move off
the PE onto the vector engines using ops that hit the DVE 2x/4x perf modes
(tensor_scalar @4x, tensor_tensor @2x; NOT scalar_tensor_tensor which runs
1x):
  - ScalarE:  tmp6 = k6*x(2,0), tmp3 = k3*x(1,0)      (activation scale)
  - DVE:      t7 = k7*x(2,1), t8 = k8*x(2,2)          (tensor_scalar, 4x)
              acc = ((t7+t8)+tmp6)+tmp3               (tensor_tensor, 2x)
  - GpSimd:   acc_d = k4*x(1,1) + acc                 (scalar_tensor_tensor)
  - PE:       per 2-row psum slice: 4 shift matmuls (taps (0,0),(0,1),(0,2),
              (1,2)) + one mix matmul blockdiag(M.T) @ acc_d.
PSUM tiles are 4 rows (2 banks) x4 bufs; the mix matmuls for group g are
deferred until after the shifts of group g+2 so the vector chain has ~2 group
periods of slack and drains (PSUM fp32 -> SBUF bf16, alternating ScalarE /
GpSimd per group) overlap later shifts instead of serializing PE. Stores
ride SP HWDGE deferred two chunks behind loads. Each chunk reloads its 2
halo rows from DRAM (cheaper than on-chip copies at bf16 DMA rates).

Sharding (8 cores): core i = (sample pair i//2, row half i%2); partitions
hold (2 samples x 64 ch); host pre-pads shards with the 1-pixel halo.
"""

import numpy as np

B, C, H, W = 8, 64, 256, 256
N_CORES = 8
HH = H // 2  # rows per core (half image)
PR, PC = HH + 2, W + 2  # padded shard rows/cols (halo)
ROWS_PER_CHUNK = 16
N_CHUNKS = HH // ROWS_PER_CHUNK
GP = 4  # rows per psum group (2 banks)
GV = 8  # rows per vector-chain op

# tap order: si = 3*dy + dx
PE_TAPS = [0, 1, 2, 5, 4]  # (0,0) (0,1) (0,2) (1,2) (1,1)
MIX = len(PE_TAPS)  # wt slot holding blockdiag(M.T)
SCA_TAP = 3  # ScalarE scale tap: (1,0)
DVE_TAPS = [6, 7, 8]  # DVE tensor_scalar taps: (2,0), (2,1), (2,2)

N_WARMUP = 8  # PE p-state warmup matmuls on a zeroed scratch tile
DEFER = 2  # steady-state groups between a group's shifts and its mix
DEFER0 = 6  # fill-phase defer (chunk 0: no psum reuse pressure yet)

LAST_EXEC_NS = None
_PROGRAM = None


def _build_program(n_warmup=N_WARMUP, defer=DEFER, defer0=DEFER0,
                   store_defer=2, memset_eng="vector", dve_tail_last=True,
                   act_tail_drains=True, sca_pieces0=2):
    import concourse.mybir as mybir
    import concourse.tile as tile
    from concourse import bacc

    f32 = mybir.dt.float32
    bf16 = mybir.dt.bfloat16
    mult = mybir.AluOpType.mult
    add = mybir.AluOpType.add
    Copy = mybir.ActivationFunctionType.Copy

    nc = bacc.Bacc(
        "TRN2", target_bir_lowering=False, debug=False, num_devices=N_CORES
    )
    x_d = nc.dram_tensor("xs", [128, PR, PC], bf16, kind="ExternalInput").ap()
    w_d = nc.dram_tensor("wt", [128, MIX + 1, 128], bf16, kind="ExternalInput").ap()
    k_d = nc.dram_tensor("kv", [128, 9], f32, kind="ExternalInput").ap()
    o_d = nc.dram_tensor("out", [128, HH, W], bf16, kind="ExternalOutput").ap()

    def dy_dx(si):
        return si // 3, si % 3

    with tile.TileContext(nc) as tc:
        with (
            tc.tile_pool(name="wpool", bufs=1) as wpool,
            tc.tile_pool(name="xpool", bufs=3) as xpool,
            tc.tile_pool(name="ppool", bufs=1, space="PSUM") as ppool,
            tc.tile_pool(name="opool", bufs=3) as opool,
            tc.tile_pool(name="vpool", bufs=2) as vpool,
            tc.tile_pool(name="apool", bufs=3) as apool,
        ):
            # One big psum tile spanning all 8 banks; matmuls write 2-row
            # slices, drains read 8-row blocks. Tile's AP-overlap tracking
            # gives each 2-row slice a ~6-group reuse slack (vs a tiled pool
            # where every slice waits the oldest drain).
            ps_all = ppool.tile([128, 16, W], f32, name="ps", tag="ps")

            # PE warm-up: dummy matmuls on a zeroed scratch tile fill the
            # initial DMA wait so the p-state is ramped when real work lands.
            # Alternate psum rows so they stream back-to-back (no WAW chain).
            scratch = wpool.tile([128, 512], f32, tag="scratch")
            getattr(nc, memset_eng).memset(scratch, 0.0)
            sc16 = scratch.bitcast(bf16)
            for wi in range(n_warmup):
                pw = 12 + 2 * (wi % 2)
                nc.tensor.matmul(
                    ps_all[:, pw : pw + 2, :], lhsT=sc16[:, :128],
                    rhs=sc16[:, :512], start=True, stop=True,
                )

            wt = wpool.tile([128, MIX + 1, 128], bf16)
            nc.scalar.dma_start(out=wt, in_=w_d)
            kv = wpool.tile([128, 9], f32)
            nc.scalar.dma_start(out=kv, in_=k_d)

            pending = []  # (group, acc_d 2-row slice, ot) awaiting mix
            drained = []  # callbacks run as tail-chunk rows drain

            def flush_one():
                g, acc_d2, ot = pending.pop(0)
                pr = (2 * g) % 16
                nc.tensor.matmul(
                    ps_all[:, pr : pr + 2, :],
                    lhsT=wt[:, MIX, :], rhs=acc_d2,
                    start=False, stop=True,
                )
                if g % 4 == 3:
                    # mixes of 8-row block b complete -> drain it.
                    # GPSIMD cannot read PSUM, so drains live on ScalarE
                    # (with DVE helping at the tail where it idles).
                    b = g // 4
                    c, hb = divmod(b, 2)
                    pb = (8 * b) % 16
                    if b >= 2 * N_CHUNKS - 2 and act_tail_drains:
                        # tail blocks: drain 4-row halves on both engines in
                        # parallel and store each half as soon as it lands
                        for q, eng in ((0, nc.vector.tensor_copy),
                                       (1, nc.scalar.copy)):
                            rq = 8 * hb + 4 * q
                            eng(out=ot[:, rq : rq + 4, :],
                                in_=ps_all[:, pb + 4 * q : pb + 4 * q + 4, :])
                            drained.append((c, ot, rq))
                    else:
                        nc.scalar.copy(
                            out=ot[:, 8 * hb : 8 * hb + 8, :],
                            in_=ps_all[:, pb : pb + 8, :],
                        )

            stores = []  # (chunk, ot) awaiting store
            for c in range(N_CHUNKS):
                r0 = ROWS_PER_CHUNK * c
                # keep the mix/drain flush ahead of this chunk's shifts so a
                # shift's psum-reuse WAR never waits on a mix queued after it
                while len(pending) > defer:
                    flush_one()
                xt = xpool.tile(
                    [128, ROWS_PER_CHUNK + 2, PC], bf16, name="xt", tag="xt"
                )
                if c == 0:
                    # split first load so PE / the chain start early
                    nc.sync.dma_start(out=xt[:, :6, :], in_=x_d[:, :6, :])
                    nc.sync.dma_start(out=xt[:, 6:12, :], in_=x_d[:, 6:12, :])
                    nc.sync.dma_start(out=xt[:, 12:, :], in_=x_d[:, 12:18, :])
                else:
                    nc.sync.dma_start(
                        out=xt, in_=x_d[:, r0 : r0 + ROWS_PER_CHUNK + 2, :]
                    )

                def xs(si, a, b):
                    dy, dx = dy_dx(si)
                    return xt[:, a + dy : b + dy, dx : dx + W]

                # Chain per 8-row half h. GPSIMD cannot read PSUM and runs no
                # TensorScalarPtr ops, so: ScalarE owns the drains + one scale
                # tap, DVE scales three taps, Pool contributes one (slow)
                # TensorTensor combine off the critical tail:
                #   DVE:  t6,t7,t8 = k_s*x_s  (tensor_scalar @4x)
                #   Act:  tmp3 = k3*x3        (activation scale)
                #   Pool: c67 = t6 + t7       (TT @0.42 eff; early inputs)
                #   DVE:  d = t8 + tmp3 ; acc_d = c67 + d
                tmp3 = vpool.tile([128, ROWS_PER_CHUNK, W], bf16, tag="tmp3")
                for a in range(0, ROWS_PER_CHUNK, GV):
                    nc.scalar.activation(
                        out=tmp3[:, a : a + GV, :],
                        in_=xs(SCA_TAP, a, a + GV), func=Copy,
                        scale=kv[:, SCA_TAP : SCA_TAP + 1],
                    )

                acc_ds = []
                for h in range(ROWS_PER_CHUNK // GV):
                    acc_d = apool.tile([128, GV, W], bf16, tag="acc_d")
                    acc_ds.append(acc_d)
                for h in range(ROWS_PER_CHUNK // GV):
                    a = GV * h
                    ts = []
                    for si in DVE_TAPS:
                        t = vpool.tile([128, GV, W], bf16, tag=f"t{si}")
                        nc.vector.tensor_scalar_mul(
                            t, xs(si, a, a + GV), kv[:, si : si + 1]
                        )
                        ts.append(t)
                    c67 = vpool.tile([128, GV, W], bf16, tag="c67")
                    if dve_tail_last and c == N_CHUNKS - 1:
                        nc.vector.tensor_add(c67, ts[0], ts[1])
                    else:
                        nc.gpsimd.tensor_add(c67, ts[0], ts[1])
                    d = vpool.tile([128, GV, W], bf16, tag="d")
                    nc.vector.tensor_add(d, ts[2], tmp3[:, a : a + GV, :])
                    nc.vector.tensor_add(acc_ds[h], c67, d)

                ot = opool.tile([128, ROWS_PER_CHUNK, W], bf16, tag="ot")
                cur_defer = defer0 if c == 0 else defer
                for gi in range(ROWS_PER_CHUNK // 2):
                    g = (ROWS_PER_CHUNK // 2) * c + gi
                    wr = 2 * gi  # row offset within chunk
                    pr = (2 * g) % 16  # row offset within the psum tile
                    for idx, si in enumerate(PE_TAPS):
                        dy, dx = dy_dx(si)
                        nc.tensor.matmul(
                            ps_all[:, pr : pr + 2, :],
                            lhsT=wt[:, idx, :],
                            rhs=xt[:, wr + dy : wr + dy + 2, dx : dx + W],
                            start=(idx == 0), stop=False,
                        )
                    while len(pending) >= cur_defer:
                        flush_one()
                    acc_d2 = acc_ds[wr // GV][:, wr % GV : wr % GV + 2, :]
                    pending.append((g, acc_d2, ot))

                stores.append((c, ot))
                if len(stores) > store_defer:
                    sc, sot = stores.pop(0)
                    nc.sync.dma_start(
                        out=o_d[:, ROWS_PER_CHUNK * sc : ROWS_PER_CHUNK * (sc + 1), :],
                        in_=sot,
                    )
            while pending:
                flush_one()
            for sc, sot in stores:
                if sc == N_CHUNKS - 1 and drained:
                    continue  # stored via the fine-grained drain pieces below
                nc.sync.dma_start(
                    out=o_d[:, ROWS_PER_CHUNK * sc : ROWS_PER_CHUNK * (sc + 1), :],
                    in_=sot,
                )
            for sc, sot, rq in drained:
                nc.sync.dma_start(
                    out=o_d[:, ROWS_PER_CHUNK * sc + rq : ROWS_PER_CHUNK * sc + rq + 4, :],
                    in_=sot[:, rq : rq + 4, :],
                )
    nc.compile()
    return nc


def _get_program():
    global _PROGRAM
    if _PROGRAM is None:
        _PROGRAM = _build_program()
    return _PROGRAM


def _softmax_rows(a):
    a = a.astype(np.float64)
    a = np.exp(a - a.max(axis=1, keepdims=True))
    return a / a.sum(axis=1, keepdims=True)


def _make_weights(depthwise_weights, pointwise_weights, attention_weights,
                  global_attention_weight):
    import ml_dtypes

    A = _softmax_rows(np.asarray(attention_weights))
    G = _softmax_rows(np.asarray(global_attention_weight))
    P = np.asarray(pointwise_weights)[:, :, 0, 0].astype(np.float64)
    M = G @ P @ A  # (64, 64): out = M @ dw per pixel
    Kdw = np.asarray(depthwise_weights)[:, 0].astype(np.float64)  # (64, 3, 3)
    wt = np.zeros((128, MIX + 1, 128), np.float32)
    for idx, si in enumerate(PE_TAPS):
        dy, dx = si // 3, si % 3
        blk = (M.T * Kdw[:, dy, dx][:, None]).astype(np.float32)  # (c, j)
        wt[:C, idx, :C] = blk
        wt[C:, idx, C:] = blk
    mixT = M.T.astype(np.float32)
    wt[:C, MIX, :C] = mixT
    wt[C:, MIX, C:] = mixT
    kva = np.empty((128, 9), np.float32)
    for si in range(9):
        dy, dx = si // 3, si % 3
        kva[:C, si] = Kdw[:, dy, dx]
        kva[C:, si] = Kdw[:, dy, dx]
    return wt.astype(ml_dtypes.bfloat16), kva


def _make_shards(x):
    import ml_dtypes

    x = np.asarray(x, dtype=np.float32)
    shards = []
    for i in range(N_CORES):
        p, h = divmod(i, 2)
        buf = np.zeros((2, C, PR, PC), ml_dtypes.bfloat16)
        r0 = HH * h - 1
        r1 = HH * h + HH + 1
        sr0, sr1 = max(r0, 0), min(r1, H)
        buf[:, :, sr0 - r0 : sr1 - r0, 1 : 1 + W] = x[
            2 * p : 2 * p + 2, :, sr0:sr1, :
        ].astype(ml_dtypes.bfloat16)
        shards.append(buf.reshape(128, PR, PC))
    return shards


def kernel(x, depthwise_weights, pointwise_weights, attention_weights,
           global_attention_weight):
    global LAST_EXEC_NS
    from concourse import bass_utils

    nc = _get_program()
    wt, kv = _make_weights(depthwise_weights, pointwise_weights,
                           attention_weights, global_attention_weight)
    shards = _make_shards(x)
    in_maps = [{"xs": shards[i], "wt": wt, "kv": kv} for i in range(N_CORES)]

    res = bass_utils.run_bass_kernel_spmd(
        nc, in_maps, core_ids=list(range(N_CORES)), trace=False
    )
    LAST_EXEC_NS = res.exec_time_ns

    out = np.empty((B, C, H, W), np.float32)
    for i in range(N_CORES):
        p, h = divmod(i, 2)
        o = res.results[i]["out"].astype(np.float32).reshape(2, C, HH, W)
        out[2 * p : 2 * p + 2, :, HH * h : HH * h + HH, :] = o
    return out


# revision 31
# speedup vs baseline: 1.0761x; 1.0129x over previous
"""Trainium2 Bass kernel for nn_DepthwiseMultiKernelAttention.

Reference computes:
    dw  = depthwise3x3(x, K)              (pad=1, stride=1, per-channel)
    out = softmax_rows(G) @ P @ softmax_rows(A) @ dw     (channel mixes)

All three channel mixes fold into one 64x64 matrix M = G_sm @ P @ A_sm, and
folding M through the depthwise conv gives nine shifted channel-mix matmuls:
    out[:, h, w] = sum_{dy,dx} M9[dy,dx] @ x[:, h+dy-1, w+dx-1]
    M9[dy,dx][j, c] = M[j, c] * K[c, dy, dx]

v2: everything is bf16 on the wire and on-chip. The host casts x to bf16
before upload and the output back to fp32 after download, halving HBM traffic
(the memory roofline) vs the fp32 baseline. Five of the nine taps move off
the PE onto the vector engines using ops that hit the DVE 2x/4x perf modes
(tensor_scalar @4x, tensor_tensor @2x; NOT scalar_tensor_tensor which runs
1x):
  - ScalarE:  tmp6 = k6*x(2,0), tmp3 = k3*x(1,0)      (activation scale)
  - DVE:      t7 = k7*x(2,1), t8 = k8*x(2,2)          (tensor_scalar, 4x)
              acc = ((t7+t8)+tmp6)+tmp3               (tensor_tensor, 2x)
  - GpSimd:   acc_d = k4*x(1,1) + acc                 (scalar_tensor_tensor)
  - PE:       per 2-row psum slice: 4 shift matmuls (taps (0,0),(0,1),(0,2),
              (1,2)) + one mix matmul blockdiag(M.T) @ acc_d.
PSUM tiles are 4 rows (2 banks) x4 bufs; the mix matmuls for group g are
deferred until after the shifts of group g+2 so the vector chain has ~2 group
periods of slack and drains (PSUM fp32 -> SBUF bf16, alternating ScalarE /
GpSimd per group) overlap later shifts instead of serializing PE. Stores
ride SP HWDGE deferred two chunks behind loads. Each chunk reloads its 2
halo rows from DRAM (cheaper than on-chip copies at bf16 DMA rates).

Sharding (8 cores): core i = (sample pair i//2, row half i%2); partitions
hold (2 samples x 64 ch); host pre-pads shards with the 1-pixel halo.
"""

import numpy as np

B, C, H, W = 8, 64, 256, 256
N_CORES = 8
HH = H // 2  # rows per core (half image)
PR, PC = HH + 2, W + 2  # padded shard rows/cols (halo)
ROWS_PER_CHUNK = 16
N_CHUNKS = HH // ROWS_PER_CHUNK
GP = 4  # rows per psum group (2 banks)
GV = 8  # rows per vector-chain op

# tap order: si = 3*dy + dx
PE_TAPS = [0, 1, 2, 5, 4]  # (0,0) (0,1) (0,2) (1,2) (1,1)
MIX = len(PE_TAPS)  # wt slot holding blockdiag(M.T)
SCA_TAP = 3  # ScalarE scale tap: (1,0)
DVE_TAPS = [6, 7, 8]  # DVE tensor_scalar taps: (2,0), (2,1), (2,2)

N_WARMUP = 8  # PE p-state warmup matmuls on a zeroed scratch tile
DEFER = 2  # steady-state groups between a group's shifts and its mix
DEFER0 = 6  # fill-phase defer (chunk 0: no psum reuse pressure yet)

LAST_EXEC_NS = None
_PROGRAM = None


def _build_program(n_warmup=N_WARMUP, defer=DEFER, defer0=DEFER0,
                   store_defer=2, memset_eng="vector", dve_tail_last=True,
                   act_tail_drains=True, sca_pieces0=2, half_tap4=True,
                   dve_c67_fill=1):
    import concourse.mybir as mybir
    import concourse.tile as tile
    from concourse import bacc

    f32 = mybir.dt.float32
    bf16 = mybir.dt.bfloat16
    mult = mybir.AluOpType.mult
    add = mybir.AluOpType.add
    Copy = mybir.ActivationFunctionType.Copy

    nc = bacc.Bacc(
        "TRN2", target_bir_lowering=False, debug=False, num_devices=N_CORES
    )
    x_d = nc.dram_tensor("xs", [128, PR, PC], bf16, kind="ExternalInput").ap()
    w_d = nc.dram_tensor("wt", [128, MIX + 1, 128], bf16, kind="ExternalInput").ap()
    k_d = nc.dram_tensor("kv", [128, 9], f32, kind="ExternalInput").ap()
    o_d = nc.dram_tensor("out", [128, HH, W], bf16, kind="ExternalOutput").ap()

    def dy_dx(si):
        return si // 3, si % 3

    with tile.TileContext(nc) as tc:
        with (
            tc.tile_pool(name="wpool", bufs=1) as wpool,
            tc.tile_pool(name="xpool", bufs=3) as xpool,
            tc.tile_pool(name="ppool", bufs=1, space="PSUM") as ppool,
            tc.tile_pool(name="opool", bufs=3) as opool,
            tc.tile_pool(name="vpool", bufs=2) as vpool,
            tc.tile_pool(name="apool", bufs=3) as apool,
        ):
            # One big psum tile spanning all 8 banks; matmuls write 2-row
            # slices, drains read 8-row blocks. Tile's AP-overlap tracking
            # gives each 2-row slice a ~6-group reuse slack (vs a tiled pool
            # where every slice waits the oldest drain).
            ps_all = ppool.tile([128, 16, W], f32, name="ps", tag="ps")

            # PE warm-up: dummy matmuls on a zeroed scratch tile fill the
            # initial DMA wait so the p-state is ramped when real work lands.
            # Alternate psum rows so they stream back-to-back (no WAW chain).
            scratch = wpool.tile([128, 512], f32, tag="scratch")
            getattr(nc, memset_eng).memset(scratch, 0.0)
            sc16 = scratch.bitcast(bf16)
            for wi in range(n_warmup):
                pw = 12 + 2 * (wi % 2)
                nc.tensor.matmul(
                    ps_all[:, pw : pw + 2, :], lhsT=sc16[:, :128],
                    rhs=sc16[:, :512], start=True, stop=True,
                )

            wt = wpool.tile([128, MIX + 1, 128], bf16)
            nc.scalar.dma_start(out=wt, in_=w_d)
            kv = wpool.tile([128, 9], f32)
            nc.scalar.dma_start(out=kv, in_=k_d)

            pending = []  # (group, acc_d 2-row slice, ot) awaiting mix
            drained = []  # callbacks run as tail-chunk rows drain

            def flush_one():
                g, acc_d2, ot = pending.pop(0)
                pr = (2 * g) % 16
                nc.tensor.matmul(
                    ps_all[:, pr : pr + 2, :],
                    lhsT=wt[:, MIX, :], rhs=acc_d2,
                    start=False, stop=True,
                )
                if g % 4 == 3:
                    # mixes of 8-row block b complete -> drain it.
                    # GPSIMD cannot read PSUM, so drains live on ScalarE
                    # (with DVE helping at the tail where it idles).
                    b = g // 4
                    c, hb = divmod(b, 2)
                    pb = (8 * b) % 16
                    if b >= 2 * N_CHUNKS - 2 and act_tail_drains:
                        # tail blocks: drain 4-row halves on both engines in
                        # parallel and store each half as soon as it lands
                        for q, eng in ((0, nc.vector.tensor_copy),
                                       (1, nc.scalar.copy)):
                            rq = 8 * hb + 4 * q
                            eng(out=ot[:, rq : rq + 4, :],
                                in_=ps_all[:, pb + 4 * q : pb + 4 * q + 4, :])
                            drained.append((c, ot, rq))
                    else:
                        nc.scalar.copy(
                            out=ot[:, 8 * hb : 8 * hb + 8, :],
                            in_=ps_all[:, pb : pb + 8, :],
                        )

            def xs_of(xt, si, a, b):
                dy, dx = dy_dx(si)
                return xt[:, a + dy : b + dy, dx : dx + W]

            def load(c):
                xt = xpool.tile(
                    [128, ROWS_PER_CHUNK + 2, PC], bf16, name="xt", tag="xt"
                )
                r0 = ROWS_PER_CHUNK * c
                if c == 0:
                    # split first load so PE / the chain start early
                    nc.sync.dma_start(out=xt[:, :6, :], in_=x_d[:, :6, :])
                    nc.sync.dma_start(out=xt[:, 6:12, :], in_=x_d[:, 6:12, :])
                    nc.sync.dma_start(out=xt[:, 12:, :], in_=x_d[:, 12:18, :])
                else:
                    nc.sync.dma_start(
                        out=xt, in_=x_d[:, r0 : r0 + ROWS_PER_CHUNK + 2, :]
                    )
                return xt

            def chain_head(xt, c):
                """DVE t6/t7 scales + Pool c67 combines for chunk c — emitted
                one iteration early so Pool's slow TTs run a half-chunk ahead
                of the mixes that transitively need them."""
                c67s = []
                for h in range(ROWS_PER_CHUNK // GV):
                    a = GV * h
                    t6 = vpool.tile([128, GV, W], bf16, tag="t6", bufs=3)
                    nc.vector.tensor_scalar_mul(
                        t6, xs_of(xt, DVE_TAPS[0], a, a + GV),
                        kv[:, DVE_TAPS[0] : DVE_TAPS[0] + 1],
                    )
                    t7 = vpool.tile([128, GV, W], bf16, tag="t7", bufs=3)
                    nc.vector.tensor_scalar_mul(
                        t7, xs_of(xt, DVE_TAPS[1], a, a + GV),
                        kv[:, DVE_TAPS[1] : DVE_TAPS[1] + 1],
                    )
                    c67 = vpool.tile([128, GV, W], bf16, tag="c67", bufs=3)
                    if c < dve_c67_fill:
                        nc.vector.tensor_add(c67, t6, t7)
                    else:
                        nc.gpsimd.tensor_add(c67, t6, t7)
                    c67s.append(c67)
                return c67s

            stores = []  # (chunk, ot) awaiting store
            xts = {0: load(0)}
            head = {0: chain_head(xts[0], 0)}
            for c in range(N_CHUNKS):
                xt = xts[c]
                # keep the mix/drain flush ahead of this chunk's shifts so a
                # shift's psum-reuse WAR never waits on a mix queued after it
                while len(pending) > defer:
                    flush_one()
                if c + 1 < N_CHUNKS:
                    xts[c + 1] = load(c + 1)

                def xs(si, a, b):
                    return xs_of(xt, si, a, b)

                # Chain tail per 8-row half h (GPSIMD cannot read PSUM and
                # runs no TensorScalarPtr ops — ScalarE owns drains + one
                # scale tap; DVE finishes the chain):
                #   Act:  tmp3 = k3*x3          (activation scale)
                #   DVE:  t8 = k8*x8 ; d = t8 + tmp3 (+ k4*x4 for h0)
                #         acc_d = c67 + d       (c67 from chain_head)
                tmp3 = vpool.tile([128, ROWS_PER_CHUNK, W], bf16, tag="tmp3")
                for a in range(0, ROWS_PER_CHUNK, GV):
                    nc.scalar.activation(
                        out=tmp3[:, a : a + GV, :],
                        in_=xs(SCA_TAP, a, a + GV), func=Copy,
                        scale=kv[:, SCA_TAP : SCA_TAP + 1],
                    )

                c67s = head.pop(c)
                acc_ds = []
                tails = {}
                for h in range(ROWS_PER_CHUNK // GV):
                    a = GV * h
                    acc_d = apool.tile([128, GV, W], bf16, tag="acc_d")
                    acc_ds.append(acc_d)
                    if half_tap4 and h == 0:
                        # h0: ScalarE also scales tap8; DVE absorbs tap (1,1)
                        # so PE runs only 4 shifts for these groups
                        tmp8 = vpool.tile([128, GV, W], bf16, tag="tmp8")
                        nc.scalar.activation(
                            out=tmp8, in_=xs(DVE_TAPS[2], a, a + GV), func=Copy,
                            scale=kv[:, DVE_TAPS[2] : DVE_TAPS[2] + 1],
                        )
                        d = vpool.tile([128, GV, W], bf16, tag="d")
                        nc.vector.tensor_add(d, tmp8, tmp3[:, a : a + GV, :])
                        t4 = vpool.tile([128, GV, W], bf16, tag="t4")
                        nc.vector.tensor_scalar_mul(
                            t4, xs(4, a, a + GV), kv[:, 4:5]
                        )
                        d2 = vpool.tile([128, GV, W], bf16, tag="d2")
                        nc.vector.tensor_add(d2, d, t4)
                        d = d2
                    else:
                        t8 = vpool.tile([128, GV, W], bf16, tag="t8")
                        nc.vector.tensor_scalar_mul(
                            t8, xs(DVE_TAPS[2], a, a + GV),
                            kv[:, DVE_TAPS[2] : DVE_TAPS[2] + 1],
                        )
                        d = vpool.tile([128, GV, W], bf16, tag="d")
                        nc.vector.tensor_add(d, t8, tmp3[:, a : a + GV, :])
                    if h == 0:
                        nc.vector.tensor_add(acc_d, c67s[h], d)
                    else:
                        tails[h] = (acc_d, c67s[h], d)
                # next chunk's chain head sits between this chunk's tail ops
                # in priority order: tails first, prefetch next
                if c + 1 < N_CHUNKS:
                    head[c + 1] = chain_head(xts[c + 1], c + 1)
                for h, (acc_d, c67, d) in tails.items():
                    nc.vector.tensor_add(acc_d, c67, d)

                ot = opool.tile([128, ROWS_PER_CHUNK, W], bf16, tag="ot")
                cur_defer = defer0 if c == 0 else defer
                for gi in range(ROWS_PER_CHUNK // 2):
                    g = (ROWS_PER_CHUNK // 2) * c + gi
                    wr = 2 * gi  # row offset within chunk
                    pr = (2 * g) % 16  # row offset within the psum tile
                    taps = PE_TAPS
                    if half_tap4 and wr < GV:
                        taps = PE_TAPS[:-1]  # tap (1,1) rides the DVE chain
                    for idx, si in enumerate(taps):
                        dy, dx = dy_dx(si)
                        nc.tensor.matmul(
                            ps_all[:, pr : pr + 2, :],
                            lhsT=wt[:, idx, :],
                            rhs=xt[:, wr + dy : wr + dy + 2, dx : dx + W],
                            start=(idx == 0), stop=False,
                        )
                    while len(pending) >= cur_defer:
                        flush_one()
                    acc_d2 = acc_ds[wr // GV][:, wr % GV : wr % GV + 2, :]
                    pending.append((g, acc_d2, ot))

                stores.append((c, ot))
                if len(stores) > store_defer:
                    sc, sot = stores.pop(0)
                    nc.sync.dma_start(
                        out=o_d[:, ROWS_PER_CHUNK * sc : ROWS_PER_CHUNK * (sc + 1), :],
                        in_=sot,
                    )
            while pending:
                flush_one()
            for sc, sot in stores:
                if sc == N_CHUNKS - 1 and drained:
                    continue  # stored via the fine-grained drain pieces below
                nc.sync.dma_start(
                    out=o_d[:, ROWS_PER_CHUNK * sc : ROWS_PER_CHUNK * (sc + 1), :],
                    in_=sot,
                )
            for sc, sot, rq in drained:
                nc.sync.dma_start(
                    out=o_d[:, ROWS_PER_CHUNK * sc + rq : ROWS_PER_CHUNK * sc + rq + 4, :],
                    in_=sot[:, rq : rq + 4, :],
                )
    nc.compile()
    return nc


def _get_program():
    global _PROGRAM
    if _PROGRAM is None:
        _PROGRAM = _build_program()
    return _PROGRAM


def _softmax_rows(a):
    a = a.astype(np.float64)
    a = np.exp(a - a.max(axis=1, keepdims=True))
    return a / a.sum(axis=1, keepdims=True)


def _make_weights(depthwise_weights, pointwise_weights, attention_weights,
                  global_attention_weight):
    import ml_dtypes

    A = _softmax_rows(np.asarray(attention_weights))
    G = _softmax_rows(np.asarray(global_attention_weight))
    P = np.asarray(pointwise_weights)[:, :, 0, 0].astype(np.float64)
    M = G @ P @ A  # (64, 64): out = M @ dw per pixel
    Kdw = np.asarray(depthwise_weights)[:, 0].astype(np.float64)  # (64, 3, 3)
    wt = np.zeros((128, MIX + 1, 128), np.float32)
    for idx, si in enumerate(PE_TAPS):
        dy, dx = si // 3, si % 3
        blk = (M.T * Kdw[:, dy, dx][:, None]).astype(np.float32)  # (c, j)
        wt[:C, idx, :C] = blk
        wt[C:, idx, C:] = blk
    mixT = M.T.astype(np.float32)
    wt[:C, MIX, :C] = mixT
    wt[C:, MIX, C:] = mixT
    kva = np.empty((128, 9), np.float32)
    for si in range(9):
        dy, dx = si // 3, si % 3
        kva[:C, si] = Kdw[:, dy, dx]
        kva[C:, si] = Kdw[:, dy, dx]
    return wt.astype(ml_dtypes.bfloat16), kva


def _make_shards(x):
    import ml_dtypes

    x = np.asarray(x, dtype=np.float32)
    shards = []
    for i in range(N_CORES):
        p, h = divmod(i, 2)
        buf = np.zeros((2, C, PR, PC), ml_dtypes.bfloat16)
        r0 = HH * h - 1
        r1 = HH * h + HH + 1
        sr0, sr1 = max(r0, 0), min(r1, H)
        buf[:, :, sr0 - r0 : sr1 - r0, 1 : 1 + W] = x[
            2 * p : 2 * p + 2, :, sr0:sr1, :
        ].astype(ml_dtypes.bfloat16)
        shards.append(buf.reshape(128, PR, PC))
    return shards


def kernel(x, depthwise_weights, pointwise_weights, attention_weights,
           global_attention_weight):
    global LAST_EXEC_NS
    from concourse import bass_utils

    nc = _get_program()
    wt, kv = _make_weights(depthwise_weights, pointwise_weights,
                           attention_weights, global_attention_weight)
    shards = _make_shards(x)
    in_maps = [{"xs": shards[i], "wt": wt, "kv": kv} for i in range(N_CORES)]

    res = bass_utils.run_bass_kernel_spmd(
        nc, in_maps, core_ids=list(range(N_CORES)), trace=False
    )
    LAST_EXEC_NS = res.exec_time_ns

    out = np.empty((B, C, H, W), np.float32)
    for i in range(N_CORES):
        p, h = divmod(i, 2)
        o = res.results[i]["out"].astype(np.float32).reshape(2, C, HH, W)
        out[2 * p : 2 * p + 2, :, HH * h : HH * h + HH, :] = o
    return out


# revision 40
# speedup vs baseline: 1.1292x; 1.0493x over previous
"""Trainium2 Bass kernel for nn_DepthwiseMultiKernelAttention.

Reference computes:
    dw  = depthwise3x3(x, K)              (pad=1, stride=1, per-channel)
    out = softmax_rows(G) @ P @ softmax_rows(A) @ dw     (channel mixes)

All three channel mixes fold into one 64x64 matrix M = G_sm @ P @ A_sm, and
folding M through the depthwise conv gives nine shifted channel-mix matmuls:
    out[:, h, w] = sum_{dy,dx} M9[dy,dx] @ x[:, h+dy-1, w+dx-1]
    M9[dy,dx][j, c] = M[j, c] * K[c, dy, dx]

v2: everything is bf16 on the wire and on-chip. The host casts x to bf16
before upload and the output back to fp32 after download, halving HBM traffic
(the memory roofline) vs the fp32 baseline. Five of the nine taps move off
the PE onto the vector engines using ops that hit the DVE 2x/4x perf modes
(tensor_scalar @4x, tensor_tensor @2x; NOT scalar_tensor_tensor which runs
1x):
  - ScalarE:  tmp6 = k6*x(2,0), tmp3 = k3*x(1,0)      (activation scale)
  - DVE:      t7 = k7*x(2,1), t8 = k8*x(2,2)          (tensor_scalar, 4x)
              acc = ((t7+t8)+tmp6)+tmp3               (tensor_tensor, 2x)
  - GpSimd:   acc_d = k4*x(1,1) + acc                 (scalar_tensor_tensor)
  - PE:       per 2-row psum slice: 4 shift matmuls (taps (0,0),(0,1),(0,2),
              (1,2)) + one mix matmul blockdiag(M.T) @ acc_d.
PSUM tiles are 4 rows (2 banks) x4 bufs; the mix matmuls for group g are
deferred until after the shifts of group g+2 so the vector chain has ~2 group
periods of slack and drains (PSUM fp32 -> SBUF bf16, alternating ScalarE /
GpSimd per group) overlap later shifts instead of serializing PE. Stores
ride SP HWDGE deferred two chunks behind loads. Each chunk reloads its 2
halo rows from DRAM (cheaper than on-chip copies at bf16 DMA rates).

Sharding (8 cores): core i = (sample pair i//2, row half i%2); partitions
hold (2 samples x 64 ch); host pre-pads shards with the 1-pixel halo.
"""

import numpy as np

B, C, H, W = 8, 64, 256, 256
N_CORES = 8
HH = H // 2  # rows per core (half image)
PR, PC = HH + 2, W + 2  # padded shard rows/cols (halo)
ROWS_PER_CHUNK = 16
N_CHUNKS = HH // ROWS_PER_CHUNK
GP = 4  # rows per psum group (2 banks)
GV = 8  # rows per vector-chain op

# tap order: si = 3*dy + dx
PE_TAPS = [0, 1, 2, 5, 4]  # (0,0) (0,1) (0,2) (1,2) (1,1)
MIX = len(PE_TAPS)  # wt slot holding blockdiag(M.T)
SCA_TAP = 3  # ScalarE scale tap: (1,0)
DVE_TAPS = [6, 7, 8]  # DVE tensor_scalar taps: (2,0), (2,1), (2,2)
VEC_TAPS = [3, 6, 7, 8]  # wt slots MIX+1.. so fill groups can run PE-only

N_WARMUP = 8  # PE p-state warmup matmuls on a zeroed scratch tile
DEFER = 2  # steady-state groups between a group's shifts and its mix
DEFER0 = 6  # fill-phase defer (chunk 0: no psum reuse pressure yet)

LAST_EXEC_NS = None
_PROGRAM = None


def _build_program(n_warmup=N_WARMUP, defer=DEFER, defer0=DEFER0,
                   store_defer=2, memset_eng="vector", dve_tail_last=True,
                   act_tail_drains=True, sca_pieces0=2, half_tap4=True,
                   dve_c67_fill=1):
    import concourse.mybir as mybir
    import concourse.tile as tile
    from concourse import bacc

    f32 = mybir.dt.float32
    bf16 = mybir.dt.bfloat16
    mult = mybir.AluOpType.mult
    add = mybir.AluOpType.add
    Copy = mybir.ActivationFunctionType.Copy

    nc = bacc.Bacc(
        "TRN2", target_bir_lowering=False, debug=False, num_devices=N_CORES
    )
    x_d = nc.dram_tensor("xs", [128, PR, PC], bf16, kind="ExternalInput").ap()
    w_d = nc.dram_tensor("wt", [128, MIX + 1 + len(VEC_TAPS), 128], bf16, kind="ExternalInput").ap()
    k_d = nc.dram_tensor("kv", [128, 9], f32, kind="ExternalInput").ap()
    o_d = nc.dram_tensor("out", [128, HH, W], bf16, kind="ExternalOutput").ap()

    def dy_dx(si):
        return si // 3, si % 3

    with tile.TileContext(nc) as tc:
        with (
            tc.tile_pool(name="wpool", bufs=1) as wpool,
            tc.tile_pool(name="xpool", bufs=3) as xpool,
            tc.tile_pool(name="ppool", bufs=1, space="PSUM") as ppool,
            tc.tile_pool(name="opool", bufs=3) as opool,
            tc.tile_pool(name="vpool", bufs=2) as vpool,
            tc.tile_pool(name="apool", bufs=3) as apool,
        ):
            # One big psum tile spanning all 8 banks; matmuls write 2-row
            # slices, drains read 8-row blocks. Tile's AP-overlap tracking
            # gives each 2-row slice a ~6-group reuse slack (vs a tiled pool
            # where every slice waits the oldest drain).
            ps_all = ppool.tile([128, 16, W], f32, name="ps", tag="ps")

            # PE warm-up: dummy matmuls on a zeroed scratch tile fill the
            # initial DMA wait so the p-state is ramped when real work lands.
            # Alternate psum rows so they stream back-to-back (no WAW chain).
            scratch = wpool.tile([128, 512], f32, tag="scratch")
            getattr(nc, memset_eng).memset(scratch, 0.0)
            sc16 = scratch.bitcast(bf16)
            for wi in range(n_warmup):
                pw = 12 + 2 * (wi % 2)
                nc.tensor.matmul(
                    ps_all[:, pw : pw + 2, :], lhsT=sc16[:, :128],
                    rhs=sc16[:, :512], start=True, stop=True,
                )

            wt = wpool.tile([128, MIX + 1 + len(VEC_TAPS), 128], bf16)
            nc.scalar.dma_start(out=wt, in_=w_d)
            kv = wpool.tile([128, 9], f32)
            nc.scalar.dma_start(out=kv, in_=k_d)

            pending = []  # (group, acc_d 2-row slice, ot) awaiting mix
            drained = []  # callbacks run as tail-chunk rows drain

            def flush_one():
                g, acc_d2, ot = pending.pop(0)
                pr = (2 * g) % 16
                if acc_d2 is not None:
                    nc.tensor.matmul(
                        ps_all[:, pr : pr + 2, :],
                        lhsT=wt[:, MIX, :], rhs=acc_d2,
                        start=False, stop=True,
                    )
                if g % 4 == 3:
                    # mixes of 8-row block b complete -> drain it.
                    # GPSIMD cannot read PSUM, so drains live on ScalarE
                    # (with DVE helping at the tail where it idles).
                    b = g // 4
                    c, hb = divmod(b, 2)
                    pb = (8 * b) % 16
                    if b >= 2 * N_CHUNKS - 2 and act_tail_drains:
                        # tail blocks: drain 4-row halves on both engines in
                        # parallel and store each half as soon as it lands
                        for q, eng in ((0, nc.vector.tensor_copy),
                                       (1, nc.scalar.copy)):
                            rq = 8 * hb + 4 * q
                            eng(out=ot[:, rq : rq + 4, :],
                                in_=ps_all[:, pb + 4 * q : pb + 4 * q + 4, :])
                            drained.append((c, ot, rq))
                    else:
                        nc.scalar.copy(
                            out=ot[:, 8 * hb : 8 * hb + 8, :],
                            in_=ps_all[:, pb : pb + 8, :],
                        )

            def xs_of(xt, si, a, b):
                dy, dx = dy_dx(si)
                return xt[:, a + dy : b + dy, dx : dx + W]

            def load(c):
                xt = xpool.tile(
                    [128, ROWS_PER_CHUNK + 2, PC], bf16, name="xt", tag="xt"
                )
                r0 = ROWS_PER_CHUNK * c
                if c == 0:
                    # split first load so PE / the chain start early
                    nc.sync.dma_start(out=xt[:, :6, :], in_=x_d[:, :6, :])
                    nc.sync.dma_start(out=xt[:, 6:12, :], in_=x_d[:, 6:12, :])
                    nc.sync.dma_start(out=xt[:, 12:, :], in_=x_d[:, 12:18, :])
                else:
                    nc.sync.dma_start(
                        out=xt, in_=x_d[:, r0 : r0 + ROWS_PER_CHUNK + 2, :]
                    )
                return xt

            def chain_head(xt, c):
                """DVE t6/t7 scales + Pool c67 combines for chunk c — emitted
                one iteration early so Pool's slow TTs run a half-chunk ahead
                of the mixes that transitively need them."""
                c67s = {}
                for h in range(ROWS_PER_CHUNK // GV):
                    if c == 0 and h == 0:
                        continue  # fill: chunk-0 h0 groups run PE-only
                    a = GV * h
                    t6 = vpool.tile([128, GV, W], bf16, tag="t6", bufs=3)
                    nc.vector.tensor_scalar_mul(
                        t6, xs_of(xt, DVE_TAPS[0], a, a + GV),
                        kv[:, DVE_TAPS[0] : DVE_TAPS[0] + 1],
                    )
                    t7 = vpool.tile([128, GV, W], bf16, tag="t7", bufs=3)
                    nc.vector.tensor_scalar_mul(
                        t7, xs_of(xt, DVE_TAPS[1], a, a + GV),
                        kv[:, DVE_TAPS[1] : DVE_TAPS[1] + 1],
                    )
                    c67 = vpool.tile([128, GV, W], bf16, tag="c67", bufs=3)
                    if c < dve_c67_fill:
                        nc.vector.tensor_add(c67, t6, t7)
                    else:
                        nc.gpsimd.tensor_add(c67, t6, t7)
                    c67s[h] = c67
                return c67s

            stores = []  # (chunk, ot) awaiting store
            xts = {0: load(0)}
            head = {0: chain_head(xts[0], 0)}
            for c in range(N_CHUNKS):
                xt = xts[c]
                # keep the mix/drain flush ahead of this chunk's shifts so a
                # shift's psum-reuse WAR never waits on a mix queued after it
                while len(pending) > defer:
                    flush_one()
                if c + 1 < N_CHUNKS:
                    xts[c + 1] = load(c + 1)

                def xs(si, a, b):
                    return xs_of(xt, si, a, b)

                # Chain tail per 8-row half h (GPSIMD cannot read PSUM and
                # runs no TensorScalarPtr ops — ScalarE owns drains + one
                # scale tap; DVE finishes the chain):
                #   Act:  tmp3 = k3*x3          (activation scale)
                #   DVE:  t8 = k8*x8 ; d = t8 + tmp3 (+ k4*x4 for h0)
                #         acc_d = c67 + d       (c67 from chain_head)
                tmp3 = vpool.tile([128, ROWS_PER_CHUNK, W], bf16, tag="tmp3")
                for a in range(0, ROWS_PER_CHUNK, GV):
                    nc.scalar.activation(
                        out=tmp3[:, a : a + GV, :],
                        in_=xs(SCA_TAP, a, a + GV), func=Copy,
                        scale=kv[:, SCA_TAP : SCA_TAP + 1],
                    )

                c67s = head.pop(c)
                acc_ds = {}
                tails = {}
                for h in range(ROWS_PER_CHUNK // GV):
                    if h not in c67s:
                        continue  # fill: chunk-0 h0 groups run PE-only
                    a = GV * h
                    acc_d = apool.tile([128, GV, W], bf16, tag="acc_d")
                    acc_ds[h] = acc_d
                    if half_tap4 and h == 0:
                        # h0: ScalarE also scales tap8; DVE absorbs tap (1,1)
                        # so PE runs only 4 shifts for these groups
                        tmp8 = vpool.tile([128, GV, W], bf16, tag="tmp8")
                        nc.scalar.activation(
                            out=tmp8, in_=xs(DVE_TAPS[2], a, a + GV), func=Copy,
                            scale=kv[:, DVE_TAPS[2] : DVE_TAPS[2] + 1],
                        )
                        d = vpool.tile([128, GV, W], bf16, tag="d")
                        nc.vector.tensor_add(d, tmp8, tmp3[:, a : a + GV, :])
                        t4 = vpool.tile([128, GV, W], bf16, tag="t4")
                        nc.vector.tensor_scalar_mul(
                            t4, xs(4, a, a + GV), kv[:, 4:5]
                        )
                        d2 = vpool.tile([128, GV, W], bf16, tag="d2")
                        nc.vector.tensor_add(d2, d, t4)
                        d = d2
                    else:
                        t8 = vpool.tile([128, GV, W], bf16, tag="t8")
                        nc.vector.tensor_scalar_mul(
                            t8, xs(DVE_TAPS[2], a, a + GV),
                            kv[:, DVE_TAPS[2] : DVE_TAPS[2] + 1],
                        )
                        d = vpool.tile([128, GV, W], bf16, tag="d")
                        nc.vector.tensor_add(d, t8, tmp3[:, a : a + GV, :])
                    if h == 0:
                        nc.vector.tensor_add(acc_d, c67s[h], d)
                    else:
                        tails[h] = (acc_d, c67s[h], d)
                # next chunk's chain head sits between this chunk's tail ops
                # in priority order: tails first, prefetch next
                if c + 1 < N_CHUNKS:
                    head[c + 1] = chain_head(xts[c + 1], c + 1)
                for h, (acc_d, c67, d) in tails.items():
                    nc.vector.tensor_add(acc_d, c67, d)

                ot = opool.tile([128, ROWS_PER_CHUNK, W], bf16, tag="ot")
                cur_defer = defer0 if c == 0 else defer
                for gi in range(ROWS_PER_CHUNK // 2):
                    g = (ROWS_PER_CHUNK // 2) * c + gi
                    wr = 2 * gi  # row offset within chunk
                    pr = (2 * g) % 16  # row offset within the psum tile
                    pe_only = wr // GV not in acc_ds
                    if pe_only:
                        # fill groups: all 9 taps on PE, no mix needed
                        taps = list(enumerate(PE_TAPS)) + [
                            (MIX + 1 + i, si) for i, si in enumerate(VEC_TAPS)
                        ]
                    elif half_tap4 and wr < GV:
                        taps = list(enumerate(PE_TAPS[:-1]))  # (1,1) on DVE
                    else:
                        taps = list(enumerate(PE_TAPS))
                    for k, (idx, si) in enumerate(taps):
                        dy, dx = dy_dx(si)
                        nc.tensor.matmul(
                            ps_all[:, pr : pr + 2, :],
                            lhsT=wt[:, idx, :],
                            rhs=xt[:, wr + dy : wr + dy + 2, dx : dx + W],
                            start=(k == 0),
                            stop=(pe_only and k == len(taps) - 1),
                        )
                    while len(pending) >= cur_defer:
                        flush_one()
                    acc_d2 = None
                    if not pe_only:
                        acc_d2 = acc_ds[wr // GV][:, wr % GV : wr % GV + 2, :]
                    pending.append((g, acc_d2, ot))

                stores.append((c, ot))
                if len(stores) > store_defer:
                    sc, sot = stores.pop(0)
                    nc.sync.dma_start(
                        out=o_d[:, ROWS_PER_CHUNK * sc : ROWS_PER_CHUNK * (sc + 1), :],
                        in_=sot,
                    )
            # earlier chunks' stores go out before the final flush so their
            # transfers don't queue ahead of the tail pieces on DMA_ENGINES
            for sc, sot in stores:
                if sc == N_CHUNKS - 1:
                    continue  # stored via the fine-grained drain pieces below
                nc.sync.dma_start(
                    out=o_d[:, ROWS_PER_CHUNK * sc : ROWS_PER_CHUNK * (sc + 1), :],
                    in_=sot,
                )
            while pending:
                flush_one()
            for sc, sot, rq in drained:
                nc.sync.dma_start(
                    out=o_d[:, ROWS_PER_CHUNK * sc + rq : ROWS_PER_CHUNK * sc + rq + 4, :],
                    in_=sot[:, rq : rq + 4, :],
                )
    nc.compile()
    return nc


def _get_program():
    global _PROGRAM
    if _PROGRAM is None:
        _PROGRAM = _build_program()
    return _PROGRAM


def _softmax_rows(a):
    a = a.astype(np.float64)
    a = np.exp(a - a.max(axis=1, keepdims=True))
    return a / a.sum(axis=1, keepdims=True)


def _make_weights(depthwise_weights, pointwise_weights, attention_weights,
                  global_attention_weight):
    import ml_dtypes

    A = _softmax_rows(np.asarray(attention_weights))
    G = _softmax_rows(np.asarray(global_attention_weight))
    P = np.asarray(pointwise_weights)[:, :, 0, 0].astype(np.float64)
    M = G @ P @ A  # (64, 64): out = M @ dw per pixel
    Kdw = np.asarray(depthwise_weights)[:, 0].astype(np.float64)  # (64, 3, 3)
    wt = np.zeros((128, MIX + 1 + len(VEC_TAPS), 128), np.float32)
    for idx, si in list(enumerate(PE_TAPS)) + [
        (MIX + 1 + i, si) for i, si in enumerate(VEC_TAPS)
    ]:
        dy, dx = si // 3, si % 3
        blk = (M.T * Kdw[:, dy, dx][:, None]).astype(np.float32)  # (c, j)
        wt[:C, idx, :C] = blk
        wt[C:, idx, C:] = blk
    mixT = M.T.astype(np.float32)
    wt[:C, MIX, :C] = mixT
    wt[C:, MIX, C:] = mixT
    kva = np.empty((128, 9), np.float32)
    for si in range(9):
        dy, dx = si // 3, si % 3
        kva[:C, si] = Kdw[:, dy, dx]
        kva[C:, si] = Kdw[:, dy, dx]
    return wt.astype(ml_dtypes.bfloat16), kva


def _make_shards(x):
    import ml_dtypes

    x = np.asarray(x, dtype=np.float32)
    shards = []
    for i in range(N_CORES):
        p, h = divmod(i, 2)
        buf = np.zeros((2, C, PR, PC), ml_dtypes.bfloat16)
        r0 = HH * h - 1
        r1 = HH * h + HH + 1
        sr0, sr1 = max(r0, 0), min(r1, H)
        buf[:, :, sr0 - r0 : sr1 - r0, 1 : 1 + W] = x[
            2 * p : 2 * p + 2, :, sr0:sr1, :
        ].astype(ml_dtypes.bfloat16)
        shards.append(buf.reshape(128, PR, PC))
    return shards


def kernel(x, depthwise_weights, pointwise_weights, attention_weights,
           global_attention_weight):
    global LAST_EXEC_NS
    from concourse import bass_utils

    nc = _get_program()
    wt, kv = _make_weights(depthwise_weights, pointwise_weights,
                           attention_weights, global_attention_weight)
    shards = _make_shards(x)
    in_maps = [{"xs": shards[i], "wt": wt, "kv": kv} for i in range(N_CORES)]

    res = bass_utils.run_bass_kernel_spmd(
        nc, in_maps, core_ids=list(range(N_CORES)), trace=False
    )
    LAST_EXEC_NS = res.exec_time_ns

    out = np.empty((B, C, H, W), np.float32)
    for i in range(N_CORES):
        p, h = divmod(i, 2)
        o = res.results[i]["out"].astype(np.float32).reshape(2, C, HH, W)
        out[2 * p : 2 * p + 2, :, HH * h : HH * h + HH, :] = o
    return out
